# revision 31
# baseline (speedup 1.0000x reference)
"""AttentiveManifoldMixer Trainium2 kernel (8-core data parallel over batch).

Math: with W3[c,i,j] = conv_w[c*64+i, j], B = conv_b.reshape(C, C),
  s[b]       = sigmoid(fc2 @ relu(fc1 @ mean_hw(x[b])))
  out[b,c,p] = sum_{i,j} W3[c,i,j] * s[b,j] * x[b,i,p] * x[b,j,p]
               + sum_i B[c,i] * x[b,i,p]

The quadratic form is symmetrized over unordered channel pairs grouped by
cyclic diagonal offset d: chunk m = 3k+l holds lanes q = 64*qhi + qlo with
  i = (qlo + a_k) % 64,   a_k = (64 - 6k) % 64
  j = (qlo + 2l + qhi) % 64
so d = j - i = 6k + 2l + qhi covers 0..33 over 17 chunks (d=32/33 lanes
duplicate at higher mult).  Per-batch weight (W3[c,i,j]s_j + W3[c,j,i]s_i)
/ mult is folded on device.

x arrives host-cast to bf16; each column half DMAs into the low rows of an
[x; x] SBUF tensor X2 whose high rows are built by an ACT copy-up that also
emits the SE channel sums.  Feature operands are 64-partition windows of
X2 materialized by direct SBUF->SBUF partition-window DMAs (two per tile:
A_k = (a_k, a_k), B_l = (2l, 2l+1); A_0 is X2 itself); the 8 copied tiles
live in one packed SBUF tensor V[128, 9, P].  GEMM: 17 bf16 matmuls
(K=128, M=64, N=512) per PSUM bank plus a K=128-padded bf16 conv_b matmul
whose rhs is the B_0 tile (lower rows = x).  The SE sigmoid vector expands
to per-lane columns with 4 consolidated DRAM gathers; the weight fold runs
on ACT with the final add on GPSIMD.  Everything pipelines in two
2048-pixel column halves across the two HWDGE rings.
"""
import sys

sys.path.insert(0, "/opt/trn_rl_repo")

import numpy as np
import ml_dtypes

B, C, H, W = 8, 64, 64, 64
P = H * W                  # 4096 pixels per sample
MID = C // 4
NCHUNK = 17                # feature chunks
NA, NB = 6, 3              # A/B window tiles; chunk m = 3*(m//3) + m%3
NSUB = 512                 # matmul free-dim subtile / psum bank columns
NSPLIT = 2                 # column halves
HALF = P // NSPLIT
NBANK = HALF // (2 * NSUB)  # psum banks per half (2 subtiles per bank)
N_CORES = 8
A_VALS = [64, 58, 52, 46, 40, 34]   # a_k row offset, k=0..5 (64 == 0 mod 64)
NT = NA + NB               # packed window tiles in V

_CACHE = {}


def _lane_maps():
    """Per-lane (i, j, mult): chunk m = 3k+l, lane q = 64*qhi + qlo:
    i = (qlo + a_k + qhi) % 64,  j = (qlo + 2l + 2*qhi) % 64."""
    i_idx = np.zeros((NCHUNK, 128), np.int64)
    j_idx = np.zeros((NCHUNK, 128), np.int64)
    for m in range(NCHUNK):
        k, l = divmod(m, 3)
        for q in range(128):
            qhi, qlo = divmod(q, 64)
            i_idx[m, q] = (qlo + A_VALS[k]) % 64
            j_idx[m, q] = (qlo + 2 * l + qhi) % 64
    lo = np.minimum(i_idx, j_idx)
    hi = np.maximum(i_idx, j_idx)
    key = lo * 64 + hi
    _, inv, counts = np.unique(key, return_inverse=True, return_counts=True)
    mult = counts[inv].reshape(key.shape).astype(np.float32)
    return i_idx, j_idx, mult


def _host_weights(conv_w, fc1_w, fc2_w):
    """Pre-gather conv_w into per-lane arrays a1/a2 of shape (128, 17, 64):
    [lane q, chunk m, out-channel c], bf16."""
    w3 = conv_w.reshape(C, C, C)  # [c, i, j]
    i_idx, j_idx, mult = _lane_maps()
    a1 = np.transpose(w3[:, i_idx, j_idx], (2, 1, 0)) / mult.T[:, :, None]
    a2 = np.transpose(w3[:, j_idx, i_idx], (2, 1, 0)) / mult.T[:, :, None]
    diag = (i_idx == j_idx).T  # [q, m]
    a2[diag] = 0.0
    fc1t = (fc1_w.T / float(P)).copy()   # (64, 16): folds the 1/HW of the mean
    fc2t = fc2_w.T.copy()                # (16, 64)
    return (np.ascontiguousarray(a1, ml_dtypes.bfloat16),
            np.ascontiguousarray(a2, ml_dtypes.bfloat16), fc1t, fc2t)


def _host_perm():
    """Permutation lhsTs [64, 9, 128] for the on-PE sigmoid expansion:
    col n<3: s[(qlo + 2n + qhi) % 64]; col 3+K: s[(qlo + 34 + 6K) % 64]."""
    pm = np.zeros((C, 9, 128), np.float32)
    q = np.arange(128)
    qhi, qlo = q // 64, q % 64
    for n in range(9):
        idx = (qlo + 2 * n + qhi) % 64 if n < NB else (qlo + 34 + 6 * (n - NB)) % 64
        pm[idx, n, q] = 1.0
    return pm


def _host_idb(conv_b):
    """conv_b as a K=128-padded bf16 lhsT [B.T; 0] for the residual matmul
    (rhs is the b=0 window tile whose lower 64 rows are x)."""
    bt = np.asarray(conv_b, np.float32).reshape(C, C).T
    idb = np.zeros((128, C), np.float32)
    idb[0:C] = bt
    return np.ascontiguousarray(idb, ml_dtypes.bfloat16)


def _build_program(niter=None):
    """Build the kernel program; with niter, wrap the body in an on-device
    For_i repeat loop (timing variant)."""
    import contextlib

    import concourse.bacc as bacc
    import concourse.bass as bass
    from concourse import mybir
    from concourse.tile import TileContext

    nc = bacc.Bacc("TRN2", target_bir_lowering=False, debug=False)
    dt = mybir.dt

    x_d = nc.dram_tensor("x", [128, P], dt.bfloat16, kind="ExternalInput")
    aw_d = nc.dram_tensor("aw", [128, 2, NCHUNK, C], dt.bfloat16,
                          kind="ExternalInput")
    f1_d = nc.dram_tensor("fc1t", [C, MID], dt.float32, kind="ExternalInput")
    f2_d = nc.dram_tensor("fc2t", [MID, C], dt.float32, kind="ExternalInput")
    id_d = nc.dram_tensor("ident", [128, C], dt.bfloat16, kind="ExternalInput")
    pm_d = nc.dram_tensor("perm", [C, 9, 128], dt.float32, kind="ExternalInput")
    out_d = nc.dram_tensor("out", [C, P], dt.float32, kind="ExternalOutput")

    with TileContext(nc) as tc:
        with tc.tile_pool(name="single", bufs=1) as single, \
             tc.tile_pool(name="dram", bufs=1, space="DRAM") as dpool, \
             tc.tile_pool(name="feat", bufs=16) as featp, \
             tc.tile_pool(name="outs", bufs=6) as outsp, \
             tc.tile_pool(name="psum", bufs=8, space="PSUM") as psum, \
             (tc.For_i(0, niter, 1,
                       hint_engines=(mybir.EngineType.PE,
                                     mybir.EngineType.DVE,
                                     mybir.EngineType.SP,
                                     mybir.EngineType.Pool,
                                     mybir.EngineType.Activation))
              if niter else contextlib.nullcontext()):

            hsls = [slice(i * HALF, (i + 1) * HALF) for i in range(NSPLIT)]
            # Two HWDGE queues: nc.sync (SP) carries the staging writes +
            # window loads + output, nc.scalar (Activation) carries x/weights
            # and the SE-expansion gathers.

            aws = single.tile([128, 2, NCHUNK, C], dt.bfloat16)
            nc.scalar.dma_start(out=aws, in_=aw_d.ap())
            a1s = aws[:, 0]
            a2s = aws[:, 1]
            f1s = single.tile([C, MID], dt.float32)
            nc.scalar.dma_start(out=f1s, in_=f1_d.ap())
            f2s = single.tile([MID, C], dt.float32)
            nc.scalar.dma_start(out=f2s, in_=f2_d.ap())
            ids = single.tile([128, C], dt.bfloat16)
            nc.scalar.dma_start(out=ids, in_=id_d.ap())
            pms = single.tile([C, 9, 128], dt.float32)
            nc.scalar.dma_start(out=pms, in_=pm_d.ap())

            # ---- prestage: cast x -> [x; x] bf16 in SBUF (X2, + per-half
            # channel sums on the low cast), then build the 9 window tiles
            # with direct SBUF->SBUF partition-window DMAs (no DRAM staging).
            # A_k window (a_k, a_k+1), a = [0, 58, 52, 46, 40, 34];
            # B_l window (2l, 2l+2).
            sums_h = [single.tile([C, 1], dt.float32, name=f"sums{h}")
                      for h in range(NSPLIT)]
            X2 = single.tile([128, P], dt.bfloat16)
            V = single.tile([128, NT, P], dt.bfloat16)
            VP = NT * P  # V per-partition pitch (elements)
            a_low = [0, 58, 52, 46, 40, 34]

            def vdst(qhi, t, hsl):
                return bass.AP(tensor=V.tensor,
                               offset=V.offset + 64 * qhi * VP + t * P
                               + hsl.start,
                               ap=[[VP, 64], [1, HALF]])

            def xwin(row0, hsl):
                return bass.AP(tensor=X2.tensor,
                               offset=X2.offset + row0 * P + hsl.start,
                               ap=[[P, 64], [1, HALF]])

            scrap = single.tile([C, HALF], dt.bfloat16)
            for h, hsl in enumerate(hsls):
                if h == 0:
                    # first half's B tiles stream straight from DRAM rows of
                    # the doubled x, in parallel with the x2 load (no serial
                    # SBUF hop in front of the first feature multiplies).
                    for l in range(NB):
                        for qhi in range(2):
                            nc.sync.dma_start(
                                out=vdst(qhi, NA + l, hsl),
                                in_=x_d.ap()[2 * l + qhi:2 * l + qhi + C,
                                             hsl])
                nc.sync.dma_start(out=X2[:, hsl], in_=x_d.ap()[:, hsl])
                if h == 0:
                    # SE sums ride a throwaway ACT copy
                    nc.scalar.activation(
                        scrap, X2[0:C, hsl],
                        mybir.ActivationFunctionType.Copy,
                        accum_out=sums_h[h])
                    order = [(k, a_low[k], a_low[k]) for k in range(1, NA)]
                else:
                    # B0-low (= x) is built by an ACT copy that also emits
                    # the SE channel sums.
                    nc.scalar.activation(
                        bass.AP(tensor=V.tensor,
                                offset=V.offset + NA * P + hsl.start,
                                ap=[[VP, C], [1, HALF]]),
                        X2[0:C, hsl],
                        mybir.ActivationFunctionType.Copy,
                        accum_out=sums_h[h])
                    nc.sync.dma_start(out=vdst(1, NA, hsl),
                                      in_=xwin(1, hsl))
                    order = [(NA + l, 2 * l, 2 * l + 1)
                             for l in range(1, NB)] + \
                        [(k, a_low[k], a_low[k]) for k in range(1, NA)]
                for t, wlo, whi in order:
                    nc.sync.dma_start(out=vdst(0, t, hsl),
                                      in_=xwin(wlo, hsl))
                    nc.sync.dma_start(out=vdst(1, t, hsl),
                                      in_=xwin(whi, hsl))

            # ---- SE path: s = sigmoid(fc2t.T @ relu(fc1t.T @ sums)) ----
            ps1 = psum.tile([MID, 1], dt.float32, tag="acc")
            for h in range(NSPLIT):
                nc.tensor.matmul(ps1, f1s, sums_h[h], start=(h == 0),
                                 stop=(h == NSPLIT - 1))
            y1 = single.tile([MID, 1], dt.float32)
            nc.scalar.activation(y1, ps1, mybir.ActivationFunctionType.Relu)
            ps2 = psum.tile([C, 1], dt.float32, tag="acc")
            nc.tensor.matmul(ps2, f2s, y1, start=True, stop=True)
            svec = single.tile([C, 1], dt.float32)
            nc.scalar.activation(svec, ps2, mybir.ActivationFunctionType.Sigmoid)

            # s-expansion on PE: column n of s12 is a permutation-matmul of
            # svec (cols 0..2: s[j] per l; cols 3..8: s[i] per K=5-k).
            sexp_ps = psum.tile([128, 9], dt.float32, tag="acc")
            for n in range(9):
                nc.tensor.matmul(sexp_ps[:, n:n + 1],
                                 pms[:, n, :], svec,
                                 start=True, stop=True)
            s12 = single.tile([128, 9], dt.float32)
            nc.scalar.copy(s12, sexp_ps)
            s1b = s12[:, 0:NB]
            s2b = s12[:, NB:NB + NA]

            # ---- fold s into weights: wc = a1*s[j] + a2*s[i] (bf16) ----
            # s1b col l serves chunks m = l (mod 3); s2b col K=5-k serves
            # chunks 3k..3k+2.  ACT muls + one GPSIMD add.
            wc = single.tile([128, NCHUNK, C], dt.bfloat16)
            t1 = single.tile([128, NCHUNK, C], dt.float32)
            t2 = single.tile([128, NCHUNK, C], dt.float32)
            for l in range(NB):
                nc.scalar.mul(t1[:, l::3, :], a1s[:, l::3, :], s1b[:, l:l + 1])
            for k in range(NA):
                ms = slice(3 * k, min(3 * k + 3, NCHUNK))
                nc.scalar.mul(t2[:, ms, :], a2s[:, ms, :],
                              s2b[:, 5 - k:6 - k])
            wcf = wc.rearrange("p a b -> p (a b)")
            t1f = t1.rearrange("p a b -> p (a b)")
            t2f = t2.rearrange("p a b -> p (a b)")
            c1, c2 = 3 * C, 9 * C
            nc.gpsimd.tensor_add(wcf[:, 0:c1], t1f[:, 0:c1], t2f[:, 0:c1])
            nc.gpsimd.tensor_add(wcf[:, c1:c2], t1f[:, c1:c2], t2f[:, c1:c2])
            nc.gpsimd.tensor_add(wcf[:, c2:], t1f[:, c2:], t2f[:, c2:])

            # ---- main sweep: per half, 17 feature TTs feed a column-tiled
            # GEMM (two 512-col subtiles run concurrently per PSUM bank).
            NSH = HALF // NSUB
            for h, hsl in enumerate(hsls):
                banks = [psum.tile([C, NSUB], dt.float32, tag="acc",
                                   name=f"bank{h}_{j}") for j in range(NSH)]
                for m in range(NCHUNK):
                    k, l = divmod(m, 3)
                    f = featp.tile([128, HALF], dt.bfloat16, tag="f")
                    ina = (X2[:, hsl] if k == 0 else
                           bass.AP(tensor=V.tensor,
                                   offset=V.offset + k * P + hsl.start,
                                   ap=[[VP, 128], [1, HALF]]))
                    nc.vector.tensor_mul(
                        f, ina,
                        bass.AP(tensor=V.tensor,
                                offset=V.offset + (NA + l) * P + hsl.start,
                                ap=[[VP, 128], [1, HALF]]))
                    for j in range(NSH):
                        nc.tensor.matmul(banks[j], wc[:, m, :],
                                         f[:, j * NSUB:(j + 1) * NSUB],
                                         start=(m == 0), stop=False)
                # conv_b term: += [B.T; 0].T @ V[b=0 slot] = B @ x (bf16)
                for j in range(NSH):
                    col = h * HALF + j * NSUB
                    nc.tensor.matmul(
                        banks[j], ids,
                        bass.AP(tensor=V.tensor,
                                offset=V.offset + NA * P + col,
                                ap=[[VP, 128], [1, NSUB]]),
                        start=False, stop=True)
                pairs = ([(0, 2), (2, 2)] if h == 0 else
                         [(0, 2), (2, 1), (3, 1)])
                for j0, nj in pairs:
                    col = h * HALF + j0 * NSUB
                    ot = outsp.tile([C, nj * NSUB], dt.float32, tag="o")
                    for jj in range(nj):
                        nc.scalar.copy(ot[:, jj * NSUB:(jj + 1) * NSUB],
                                       banks[j0 + jj])
                    nc.scalar.dma_start(
                        out=out_d.ap()[:, col:col + nj * NSUB], in_=ot)

    nc.compile()
    return nc


def _get_program(niter=None):
    key = ("nc", niter)
    if key not in _CACHE:
        _CACHE[key] = _build_program(niter)
    return _CACHE[key]


def kernel(x, fc1_w, fc2_w, conv_w, conv_b):
    from concourse.bass_utils import run_bass_kernel_spmd

    x = np.asarray(x, np.float32)
    a1, a2, fc1t, fc2t = _host_weights(
        np.asarray(conv_w, np.float32), np.asarray(fc1_w, np.float32),
        np.asarray(fc2_w, np.float32))
    # conv_b contributes sum_i B[c,i]*x_i with B = conv_b.reshape(C, C); the
    # "residual" matmul realizes it with lhsT = [B.T; 0] (identity-init -> +x).
    ident = _host_idb(conv_b)
    nc = _get_program()
    aw = np.ascontiguousarray(np.stack([a1, a2], axis=1))
    in_maps = []
    for b in range(N_CORES):
        xbb2 = x[b].reshape(C, P).astype(ml_dtypes.bfloat16)
        in_maps.append({
            "x": np.ascontiguousarray(np.concatenate([xbb2, xbb2], 0)),
            "aw": aw, "fc1t": fc1t, "fc2t": fc2t, "ident": ident,
            "perm": _host_perm(),
        })
    res = run_bass_kernel_spmd(nc, in_maps, core_ids=list(range(N_CORES)))
    out = np.stack([res.results[b]["out"].reshape(C, H, W)
                    for b in range(N_CORES)], axis=0)
    return out.astype(np.float32)


# revision 32
# speedup vs baseline: 1.0973x; 1.0973x over previous
"""AttentiveManifoldMixer Trainium2 kernel (8-core data parallel over batch).

Math: with W3[c,i,j] = conv_w[c*64+i, j], B = conv_b.reshape(C, C),
  s[b]       = sigmoid(fc2 @ relu(fc1 @ mean_hw(x[b])))
  out[b,c,p] = sum_{i,j} W3[c,i,j] * s[b,j] * x[b,i,p] * x[b,j,p]
               + sum_i B[c,i] * x[b,i,p]

The quadratic form is symmetrized over unordered channel pairs grouped by
cyclic diagonal offset d: chunk m = 3k+l holds lanes q = 64*qhi + qlo with
  i = (qlo + a_k) % 64,   a_k = (64 - 6k) % 64
  j = (qlo + 2l + qhi) % 64
so d = j - i = 6k + 2l + qhi covers 0..33 over 17 chunks (d=32/33 lanes
duplicate at higher mult).  Per-batch weight (W3[c,i,j]s_j + W3[c,j,i]s_i)
/ mult is folded on device.

x arrives host-cast to bf16; each column half DMAs into the low rows of an
[x; x] SBUF tensor X2 whose high rows are built by an ACT copy-up that also
emits the SE channel sums.  Feature operands are 64-partition windows of
X2 materialized by direct SBUF->SBUF partition-window DMAs (two per tile:
A_k = (a_k, a_k), B_l = (2l, 2l+1); A_0 is X2 itself); the 8 copied tiles
live in one packed SBUF tensor V[128, 9, P].  GEMM: 17 bf16 matmuls
(K=128, M=64, N=512) per PSUM bank plus a K=128-padded bf16 conv_b matmul
whose rhs is the B_0 tile (lower rows = x).  The SE sigmoid vector expands
to per-lane columns with 4 consolidated DRAM gathers; the weight fold runs
on ACT with the final add on GPSIMD.  Everything pipelines in two
2048-pixel column halves across the two HWDGE rings.
"""
import sys

sys.path.insert(0, "/opt/trn_rl_repo")

import numpy as np
import ml_dtypes

B, C, H, W = 8, 64, 64, 64
P = H * W                  # 4096 pixels per sample
MID = C // 4
NCHUNK = 17                # feature chunks
NA, NB = 6, 3              # A/B window tiles; chunk m = 3*(m//3) + m%3
NSUB = 512                 # matmul free-dim subtile / psum bank columns
NSPLIT = 2                 # column halves
HALF = P // NSPLIT
NBANK = HALF // (2 * NSUB)  # psum banks per half (2 subtiles per bank)
N_CORES = 8
A_VALS = [64, 58, 52, 46, 40, 34]   # a_k row offset, k=0..5 (64 == 0 mod 64)
NT = NA + NB               # packed window tiles in V

_CACHE = {}


def _lane_maps():
    """Per-lane (i, j, mult): chunk m = 3k+l, lane q = 64*qhi + qlo:
    i = (qlo + a_k + qhi) % 64,  j = (qlo + 2l + 2*qhi) % 64."""
    i_idx = np.zeros((NCHUNK, 128), np.int64)
    j_idx = np.zeros((NCHUNK, 128), np.int64)
    for m in range(NCHUNK):
        k, l = divmod(m, 3)
        for q in range(128):
            qhi, qlo = divmod(q, 64)
            i_idx[m, q] = (qlo + A_VALS[k]) % 64
            j_idx[m, q] = (qlo + 2 * l + qhi) % 64
    lo = np.minimum(i_idx, j_idx)
    hi = np.maximum(i_idx, j_idx)
    key = lo * 64 + hi
    _, inv, counts = np.unique(key, return_inverse=True, return_counts=True)
    mult = counts[inv].reshape(key.shape).astype(np.float32)
    return i_idx, j_idx, mult


def _host_weights(conv_w, fc1_w, fc2_w):
    """Pre-gather conv_w into per-lane arrays a1/a2 of shape (128, 17, 64):
    [lane q, chunk m, out-channel c], bf16."""
    w3 = conv_w.reshape(C, C, C)  # [c, i, j]
    i_idx, j_idx, mult = _lane_maps()
    a1 = np.transpose(w3[:, i_idx, j_idx], (2, 1, 0)) / mult.T[:, :, None]
    a2 = np.transpose(w3[:, j_idx, i_idx], (2, 1, 0)) / mult.T[:, :, None]
    diag = (i_idx == j_idx).T  # [q, m]
    a2[diag] = 0.0
    fc1t = (fc1_w.T / float(P)).copy()   # (64, 16): folds the 1/HW of the mean
    fc2t = fc2_w.T.copy()                # (16, 64)
    return (np.ascontiguousarray(a1, ml_dtypes.bfloat16),
            np.ascontiguousarray(a2, ml_dtypes.bfloat16), fc1t, fc2t)


def _host_perm():
    """Permutation lhsTs [64, 9, 128] for the on-PE sigmoid expansion:
    col n<3: s[(qlo + 2n + qhi) % 64]; col 3+K: s[(qlo + 34 + 6K) % 64]."""
    pm = np.zeros((C, 9, 128), np.float32)
    q = np.arange(128)
    qhi, qlo = q // 64, q % 64
    for n in range(9):
        idx = (qlo + 2 * n + qhi) % 64 if n < NB else (qlo + 34 + 6 * (n - NB)) % 64
        pm[idx, n, q] = 1.0
    return pm


def _host_idb(conv_b):
    """conv_b as a K=128-padded bf16 lhsT [B.T; 0] for the residual matmul
    (rhs is the b=0 window tile whose lower 64 rows are x)."""
    bt = np.asarray(conv_b, np.float32).reshape(C, C).T
    idb = np.zeros((128, C), np.float32)
    idb[0:C] = bt
    return np.ascontiguousarray(idb, ml_dtypes.bfloat16)


def _build_program(niter=None):
    """Build the kernel program; with niter, wrap the body in an on-device
    For_i repeat loop (timing variant)."""
    import contextlib

    import concourse.bacc as bacc
    import concourse.bass as bass
    from concourse import mybir
    from concourse.tile import TileContext

    nc = bacc.Bacc("TRN2", target_bir_lowering=False, debug=False)
    dt = mybir.dt

    x_d = nc.dram_tensor("x", [128, P], dt.bfloat16, kind="ExternalInput")
    aw_d = nc.dram_tensor("aw", [128, 2, NCHUNK, C], dt.bfloat16,
                          kind="ExternalInput")
    f1_d = nc.dram_tensor("fc1t", [C, MID], dt.float32, kind="ExternalInput")
    f2_d = nc.dram_tensor("fc2t", [MID, C], dt.float32, kind="ExternalInput")
    id_d = nc.dram_tensor("ident", [128, C], dt.bfloat16, kind="ExternalInput")
    pm_d = nc.dram_tensor("perm", [C, 9, 128], dt.float32, kind="ExternalInput")
    out_d = nc.dram_tensor("out", [C, P], dt.float32, kind="ExternalOutput")

    with TileContext(nc) as tc:
        with tc.tile_pool(name="single", bufs=1) as single, \
             tc.tile_pool(name="dram", bufs=1, space="DRAM") as dpool, \
             tc.tile_pool(name="feat", bufs=16) as featp, \
             tc.tile_pool(name="outs", bufs=6) as outsp, \
             tc.tile_pool(name="psum", bufs=8, space="PSUM") as psum, \
             (tc.For_i(0, niter, 1,
                       hint_engines=(mybir.EngineType.PE,
                                     mybir.EngineType.DVE,
                                     mybir.EngineType.SP,
                                     mybir.EngineType.Pool,
                                     mybir.EngineType.Activation))
              if niter else contextlib.nullcontext()):

            hsls = [slice(i * HALF, (i + 1) * HALF) for i in range(NSPLIT)]
            # Two HWDGE queues: nc.sync (SP) carries the staging writes +
            # window loads + output, nc.scalar (Activation) carries x/weights
            # and the SE-expansion gathers.

            aws = single.tile([128, 2, NCHUNK, C], dt.bfloat16)
            nc.scalar.dma_start(out=aws, in_=aw_d.ap())
            a1s = aws[:, 0]
            a2s = aws[:, 1]
            f1s = single.tile([C, MID], dt.float32)
            nc.scalar.dma_start(out=f1s, in_=f1_d.ap())
            f2s = single.tile([MID, C], dt.float32)
            nc.scalar.dma_start(out=f2s, in_=f2_d.ap())
            ids = single.tile([128, C], dt.bfloat16)
            nc.scalar.dma_start(out=ids, in_=id_d.ap())
            pms = single.tile([C, 9, 128], dt.float32)
            nc.scalar.dma_start(out=pms, in_=pm_d.ap())

            # ---- prestage: cast x -> [x; x] bf16 in SBUF (X2, + per-half
            # channel sums on the low cast), then build the 9 window tiles
            # with direct SBUF->SBUF partition-window DMAs (no DRAM staging).
            # A_k window (a_k, a_k+1), a = [0, 58, 52, 46, 40, 34];
            # B_l window (2l, 2l+2).
            sums_h = [single.tile([C, 1], dt.float32, name=f"sums{h}")
                      for h in range(NSPLIT)]
            X2 = single.tile([128, P], dt.bfloat16)
            V = single.tile([128, NT, P], dt.bfloat16)
            VP = NT * P  # V per-partition pitch (elements)
            a_low = [0, 58, 52, 46, 40, 34]

            def vdst(qhi, t, hsl):
                return bass.AP(tensor=V.tensor,
                               offset=V.offset + 64 * qhi * VP + t * P
                               + hsl.start,
                               ap=[[VP, 64], [1, HALF]])

            def xwin(row0, hsl):
                return bass.AP(tensor=X2.tensor,
                               offset=X2.offset + row0 * P + hsl.start,
                               ap=[[P, 64], [1, HALF]])

            for h, hsl in enumerate(hsls):
                nc.sync.dma_start(out=X2[:, hsl], in_=x_d.ap()[:, hsl])
                # B0-low (= x) is built by an ACT copy that also emits the
                # SE channel sums; everything else is a window DMA off X2.
                nc.scalar.activation(
                    bass.AP(tensor=V.tensor,
                            offset=V.offset + NA * P + hsl.start,
                            ap=[[VP, C], [1, HALF]]),
                    X2[0:C, hsl],
                    mybir.ActivationFunctionType.Copy,
                    accum_out=sums_h[h])
                nc.sync.dma_start(out=vdst(1, NA, hsl), in_=xwin(1, hsl))
                order = [(NA + l, 2 * l, 2 * l + 1) for l in range(1, NB)] + \
                    [(k, a_low[k], a_low[k]) for k in range(1, NA)]
                for t, wlo, whi in order:
                    nc.sync.dma_start(out=vdst(0, t, hsl),
                                      in_=xwin(wlo, hsl))
                    nc.sync.dma_start(out=vdst(1, t, hsl),
                                      in_=xwin(whi, hsl))

            # ---- SE path: s = sigmoid(fc2t.T @ relu(fc1t.T @ sums)) ----
            ps1 = psum.tile([MID, 1], dt.float32, tag="acc")
            for h in range(NSPLIT):
                nc.tensor.matmul(ps1, f1s, sums_h[h], start=(h == 0),
                                 stop=(h == NSPLIT - 1))
            y1 = single.tile([MID, 1], dt.float32)
            nc.scalar.activation(y1, ps1, mybir.ActivationFunctionType.Relu)
            ps2 = psum.tile([C, 1], dt.float32, tag="acc")
            nc.tensor.matmul(ps2, f2s, y1, start=True, stop=True)
            svec = single.tile([C, 1], dt.float32)
            nc.scalar.activation(svec, ps2, mybir.ActivationFunctionType.Sigmoid)

            # s-expansion on PE: column n of s12 is a permutation-matmul of
            # svec (cols 0..2: s[j] per l; cols 3..8: s[i] per K=5-k).
            sexp_ps = psum.tile([128, 9], dt.float32, tag="acc")
            for n in range(9):
                nc.tensor.matmul(sexp_ps[:, n:n + 1],
                                 pms[:, n, :], svec,
                                 start=True, stop=True)
            s12 = single.tile([128, 9], dt.float32)
            nc.scalar.copy(s12, sexp_ps)
            s1b = s12[:, 0:NB]
            s2b = s12[:, NB:NB + NA]

            # ---- fold s into weights: wc = a1*s[j] + a2*s[i] (bf16) ----
            # s1b col l serves chunks m = l (mod 3); s2b col K=5-k serves
            # chunks 3k..3k+2.  ACT muls + one GPSIMD add.
            wc = single.tile([128, NCHUNK, C], dt.bfloat16)
            t1 = single.tile([128, NCHUNK, C], dt.float32)
            t2 = single.tile([128, NCHUNK, C], dt.float32)
            for l in range(NB):
                nc.scalar.mul(t1[:, l::3, :], a1s[:, l::3, :], s1b[:, l:l + 1])
            for k in range(NA):
                ms = slice(3 * k, min(3 * k + 3, NCHUNK))
                nc.scalar.mul(t2[:, ms, :], a2s[:, ms, :],
                              s2b[:, 5 - k:6 - k])
            wcf = wc.rearrange("p a b -> p (a b)")
            t1f = t1.rearrange("p a b -> p (a b)")
            t2f = t2.rearrange("p a b -> p (a b)")
            c1, c2 = 3 * C, 9 * C
            nc.gpsimd.tensor_add(wcf[:, 0:c1], t1f[:, 0:c1], t2f[:, 0:c1])
            nc.gpsimd.tensor_add(wcf[:, c1:c2], t1f[:, c1:c2], t2f[:, c1:c2])
            nc.gpsimd.tensor_add(wcf[:, c2:], t1f[:, c2:], t2f[:, c2:])

            # ---- main sweep: per half, 17 feature TTs feed a column-tiled
            # GEMM (two 512-col subtiles run concurrently per PSUM bank).
            NSH = HALF // NSUB
            for h, hsl in enumerate(hsls):
                banks = [psum.tile([C, NSUB], dt.float32, tag="acc",
                                   name=f"bank{h}_{j}") for j in range(NSH)]
                for m in range(NCHUNK):
                    k, l = divmod(m, 3)
                    f = featp.tile([128, HALF], dt.bfloat16, tag="f")
                    ina = (X2[:, hsl] if k == 0 else
                           bass.AP(tensor=V.tensor,
                                   offset=V.offset + k * P + hsl.start,
                                   ap=[[VP, 128], [1, HALF]]))
                    nc.vector.tensor_mul(
                        f, ina,
                        bass.AP(tensor=V.tensor,
                                offset=V.offset + (NA + l) * P + hsl.start,
                                ap=[[VP, 128], [1, HALF]]))
                    for j in range(NSH):
                        nc.tensor.matmul(banks[j], wc[:, m, :],
                                         f[:, j * NSUB:(j + 1) * NSUB],
                                         start=(m == 0), stop=False)
                # conv_b term: += [B.T; 0].T @ V[b=0 slot] = B @ x (bf16)
                for j in range(NSH):
                    col = h * HALF + j * NSUB
                    nc.tensor.matmul(
                        banks[j], ids,
                        bass.AP(tensor=V.tensor,
                                offset=V.offset + NA * P + col,
                                ap=[[VP, 128], [1, NSUB]]),
                        start=False, stop=True)
                pairs = ([(0, 2), (2, 2)] if h == 0 else
                         [(0, 2), (2, 1), (3, 1)])
                for j0, nj in pairs:
                    col = h * HALF + j0 * NSUB
                    ot = outsp.tile([C, nj * NSUB], dt.float32, tag="o")
                    for jj in range(nj):
                        nc.scalar.copy(ot[:, jj * NSUB:(jj + 1) * NSUB],
                                       banks[j0 + jj])
                    nc.scalar.dma_start(
                        out=out_d.ap()[:, col:col + nj * NSUB], in_=ot)

    nc.compile()
    return nc


def _get_program(niter=None):
    key = ("nc", niter)
    if key not in _CACHE:
        _CACHE[key] = _build_program(niter)
    return _CACHE[key]


def kernel(x, fc1_w, fc2_w, conv_w, conv_b):
    from concourse.bass_utils import run_bass_kernel_spmd

    x = np.asarray(x, np.float32)
    a1, a2, fc1t, fc2t = _host_weights(
        np.asarray(conv_w, np.float32), np.asarray(fc1_w, np.float32),
        np.asarray(fc2_w, np.float32))
    # conv_b contributes sum_i B[c,i]*x_i with B = conv_b.reshape(C, C); the
    # "residual" matmul realizes it with lhsT = [B.T; 0] (identity-init -> +x).
    ident = _host_idb(conv_b)
    nc = _get_program()
    aw = np.ascontiguousarray(np.stack([a1, a2], axis=1))
    in_maps = []
    for b in range(N_CORES):
        xbb2 = x[b].reshape(C, P).astype(ml_dtypes.bfloat16)
        in_maps.append({
            "x": np.ascontiguousarray(np.concatenate([xbb2, xbb2], 0)),
            "aw": aw, "fc1t": fc1t, "fc2t": fc2t, "ident": ident,
            "perm": _host_perm(),
        })
    res = run_bass_kernel_spmd(nc, in_maps, core_ids=list(range(N_CORES)))
    out = np.stack([res.results[b]["out"].reshape(C, H, W)
                    for b in range(N_CORES)], axis=0)
    return out.astype(np.float32)


# revision 33
# speedup vs baseline: 1.2573x; 1.1458x over previous
"""AttentiveManifoldMixer Trainium2 kernel (8-core data parallel over batch).

Math: with W3[c,i,j] = conv_w[c*64+i, j], B = conv_b.reshape(C, C),
  s[b]       = sigmoid(fc2 @ relu(fc1 @ mean_hw(x[b])))
  out[b,c,p] = sum_{i,j} W3[c,i,j] * s[b,j] * x[b,i,p] * x[b,j,p]
               + sum_i B[c,i] * x[b,i,p]

The quadratic form is symmetrized over unordered channel pairs grouped by
cyclic diagonal offset d: chunk m = 3k+l holds lanes q = 64*qhi + qlo with
  i = (qlo + a_k) % 64,   a_k = (64 - 6k) % 64
  j = (qlo + 2l + qhi) % 64
so d = j - i = 6k + 2l + qhi covers 0..33 over 17 chunks (d=32/33 lanes
duplicate at higher mult).  Per-batch weight (W3[c,i,j]s_j + W3[c,j,i]s_i)
/ mult is folded on device.

x arrives host-cast to bf16 and pre-doubled ([x; x], 128 rows); each
column half DMAs it into an SBUF tensor X2 at the head of the SP HWDGE
ring.  Feature operands are 64-partition windows of X2 materialized by
direct SBUF->SBUF partition-window DMAs (two per tile: A_k = (a_k, a_k),
B_l = (2l, 2l+1); A_0 is X2 itself; B_0's low half is an ACT copy of x
that doubles as the SE channel-sum reduction); the copied tiles live in
one packed SBUF tensor V[128, 9, P].  GEMM: 17 bf16 matmuls (K=128, M=64,
N=512) per PSUM bank plus a K=128-padded bf16 conv_b matmul whose rhs is
the B_0 tile (lower rows = x).  The SE sigmoid vector expands to per-lane
columns with 9 tiny on-PE permutation matmuls (no DRAM round-trip); the
weight fold runs on ACT with a 3-way-split final add on GPSIMD so early
chunks' weights unblock the GEMM sooner.  Everything pipelines in two
2048-pixel column halves across the two HWDGE rings.
"""
import sys

sys.path.insert(0, "/opt/trn_rl_repo")

import numpy as np
import ml_dtypes

B, C, H, W = 8, 64, 64, 64
P = H * W                  # 4096 pixels per sample
MID = C // 4
NCHUNK = 17                # feature chunks
NA, NB = 6, 3              # A/B window tiles; chunk m = 3*(m//3) + m%3
NSUB = 512                 # matmul free-dim subtile / psum bank columns
NSPLIT = 2                 # column halves
HALF = P // NSPLIT
NBANK = HALF // (2 * NSUB)  # psum banks per half (2 subtiles per bank)
N_CORES = 8
A_VALS = [64, 58, 52, 46, 40, 34]   # a_k row offset, k=0..5 (64 == 0 mod 64)
NT = NA + NB               # packed window tiles in V

_CACHE = {}


def _lane_maps():
    """Per-lane (i, j, mult): chunk m = 3k+l, lane q = 64*qhi + qlo:
    i = (qlo + a_k) % 64,  j = (qlo + 2l + qhi) % 64."""
    i_idx = np.zeros((NCHUNK, 128), np.int64)
    j_idx = np.zeros((NCHUNK, 128), np.int64)
    for m in range(NCHUNK):
        k, l = divmod(m, 3)
        for q in range(128):
            qhi, qlo = divmod(q, 64)
            i_idx[m, q] = (qlo + A_VALS[k]) % 64
            j_idx[m, q] = (qlo + 2 * l + qhi) % 64
    lo = np.minimum(i_idx, j_idx)
    hi = np.maximum(i_idx, j_idx)
    key = lo * 64 + hi
    _, inv, counts = np.unique(key, return_inverse=True, return_counts=True)
    mult = counts[inv].reshape(key.shape).astype(np.float32)
    return i_idx, j_idx, mult


def _host_weights(conv_w, fc1_w, fc2_w):
    """Pre-gather conv_w into per-lane arrays a1/a2 of shape (128, 17, 64):
    [lane q, chunk m, out-channel c], bf16."""
    w3 = conv_w.reshape(C, C, C)  # [c, i, j]
    i_idx, j_idx, mult = _lane_maps()
    a1 = np.transpose(w3[:, i_idx, j_idx], (2, 1, 0)) / mult.T[:, :, None]
    a2 = np.transpose(w3[:, j_idx, i_idx], (2, 1, 0)) / mult.T[:, :, None]
    diag = (i_idx == j_idx).T  # [q, m]
    a2[diag] = 0.0
    fc1t = (fc1_w.T / float(P)).copy()   # (64, 16): folds the 1/HW of the mean
    fc2t = fc2_w.T.copy()                # (16, 64)
    return (np.ascontiguousarray(a1, ml_dtypes.bfloat16),
            np.ascontiguousarray(a2, ml_dtypes.bfloat16), fc1t, fc2t)


def _host_perm():
    """Permutation lhsTs [64, 9, 128] for the on-PE sigmoid expansion:
    col n<3: s[(qlo + 2n + qhi) % 64]; col 3+K: s[(qlo + 34 + 6K) % 64]."""
    pm = np.zeros((C, 9, 128), np.float32)
    q = np.arange(128)
    qhi, qlo = q // 64, q % 64
    for n in range(9):
        idx = (qlo + 2 * n + qhi) % 64 if n < NB else (qlo + 34 + 6 * (n - NB)) % 64
        pm[idx, n, q] = 1.0
    return pm


def _host_idb(conv_b):
    """conv_b as a K=128-padded bf16 lhsT [B.T; 0] for the residual matmul
    (rhs is the b=0 window tile whose lower 64 rows are x)."""
    bt = np.asarray(conv_b, np.float32).reshape(C, C).T
    idb = np.zeros((128, C), np.float32)
    idb[0:C] = bt
    return np.ascontiguousarray(idb, ml_dtypes.bfloat16)


def _build_program(niter=None):
    """Build the kernel program; with niter, wrap the body in an on-device
    For_i repeat loop (timing variant)."""
    import contextlib

    import concourse.bacc as bacc
    import concourse.bass as bass
    from concourse import mybir
    from concourse.tile import TileContext

    nc = bacc.Bacc("TRN2", target_bir_lowering=False, debug=False)
    dt = mybir.dt

    x_d = nc.dram_tensor("x", [128, P], dt.bfloat16, kind="ExternalInput")
    aw_d = nc.dram_tensor("aw", [128, 2, NCHUNK, C], dt.bfloat16,
                          kind="ExternalInput")
    f1_d = nc.dram_tensor("fc1t", [C, MID], dt.float32, kind="ExternalInput")
    f2_d = nc.dram_tensor("fc2t", [MID, C], dt.float32, kind="ExternalInput")
    id_d = nc.dram_tensor("ident", [128, C], dt.bfloat16, kind="ExternalInput")
    pm_d = nc.dram_tensor("perm", [C, 9, 128], dt.float32, kind="ExternalInput")
    out_d = nc.dram_tensor("out", [C, P], dt.float32, kind="ExternalOutput")

    with TileContext(nc) as tc:
        with tc.tile_pool(name="single", bufs=1) as single, \
             tc.tile_pool(name="dram", bufs=1, space="DRAM") as dpool, \
             tc.tile_pool(name="feat", bufs=16) as featp, \
             tc.tile_pool(name="outs", bufs=6) as outsp, \
             tc.tile_pool(name="psum", bufs=8, space="PSUM") as psum, \
             (tc.For_i(0, niter, 1,
                       hint_engines=(mybir.EngineType.PE,
                                     mybir.EngineType.DVE,
                                     mybir.EngineType.SP,
                                     mybir.EngineType.Pool,
                                     mybir.EngineType.Activation))
              if niter else contextlib.nullcontext()):

            hsls = [slice(i * HALF, (i + 1) * HALF) for i in range(NSPLIT)]
            # Two HWDGE queues: nc.sync (SP) carries the staging writes +
            # window loads + output, nc.scalar (Activation) carries x/weights
            # and the SE-expansion gathers.

            aws = single.tile([128, 2, NCHUNK, C], dt.bfloat16)
            nc.scalar.dma_start(out=aws, in_=aw_d.ap())
            a1s = aws[:, 0]
            a2s = aws[:, 1]
            f1s = single.tile([C, MID], dt.float32)
            nc.scalar.dma_start(out=f1s, in_=f1_d.ap())
            f2s = single.tile([MID, C], dt.float32)
            nc.scalar.dma_start(out=f2s, in_=f2_d.ap())
            ids = single.tile([128, C], dt.bfloat16)
            nc.scalar.dma_start(out=ids, in_=id_d.ap())
            pms = single.tile([C, 9, 128], dt.float32)
            nc.scalar.dma_start(out=pms, in_=pm_d.ap())

            # ---- prestage: load host-doubled [x; x] bf16 (X2), then build
            # the window tiles with direct SBUF->SBUF partition-window DMAs
            # (no DRAM staging).  A_k window (a_k, a_k), a_k in a_low;
            # B_l window (2l, 2l+1); A_0 is X2 itself.
            sums_h = [single.tile([C, 1], dt.float32, name=f"sums{h}")
                      for h in range(NSPLIT)]
            X2 = single.tile([128, P], dt.bfloat16)
            V = single.tile([128, NT, P], dt.bfloat16)
            VP = NT * P  # V per-partition pitch (elements)
            a_low = [0, 58, 52, 46, 40, 34]

            def vdst(qhi, t, hsl):
                return bass.AP(tensor=V.tensor,
                               offset=V.offset + 64 * qhi * VP + t * P
                               + hsl.start,
                               ap=[[VP, 64], [1, HALF]])

            def xwin(row0, hsl):
                return bass.AP(tensor=X2.tensor,
                               offset=X2.offset + row0 * P + hsl.start,
                               ap=[[P, 64], [1, HALF]])

            for h, hsl in enumerate(hsls):
                nc.sync.dma_start(out=X2[:, hsl], in_=x_d.ap()[:, hsl])
                # B0-low (= x) is built by an ACT copy that also emits the
                # SE channel sums; everything else is a window DMA off X2.
                nc.scalar.activation(
                    bass.AP(tensor=V.tensor,
                            offset=V.offset + NA * P + hsl.start,
                            ap=[[VP, C], [1, HALF]]),
                    X2[0:C, hsl],
                    mybir.ActivationFunctionType.Copy,
                    accum_out=sums_h[h])
                nc.sync.dma_start(out=vdst(1, NA, hsl), in_=xwin(1, hsl))
                order = [(NA + l, 2 * l, 2 * l + 1) for l in range(1, NB)] + \
                    [(k, a_low[k], a_low[k]) for k in range(1, NA)]
                for t, wlo, whi in order:
                    nc.sync.dma_start(out=vdst(0, t, hsl),
                                      in_=xwin(wlo, hsl))
                    nc.sync.dma_start(out=vdst(1, t, hsl),
                                      in_=xwin(whi, hsl))

            # ---- SE path: s = sigmoid(fc2t.T @ relu(fc1t.T @ sums)) ----
            ps1 = psum.tile([MID, 1], dt.float32, tag="acc")
            for h in range(NSPLIT):
                nc.tensor.matmul(ps1, f1s, sums_h[h], start=(h == 0),
                                 stop=(h == NSPLIT - 1))
            y1 = single.tile([MID, 1], dt.float32)
            nc.scalar.activation(y1, ps1, mybir.ActivationFunctionType.Relu)
            ps2 = psum.tile([C, 1], dt.float32, tag="acc")
            nc.tensor.matmul(ps2, f2s, y1, start=True, stop=True)
            svec = single.tile([C, 1], dt.float32)
            nc.scalar.activation(svec, ps2, mybir.ActivationFunctionType.Sigmoid)

            # s-expansion on PE: column n of s12 is a permutation-matmul of
            # svec (cols 0..2: s[j] per l; cols 3..8: s[i] per K=5-k).
            sexp_ps = psum.tile([128, 9], dt.float32, tag="acc")
            for n in range(9):
                nc.tensor.matmul(sexp_ps[:, n:n + 1],
                                 pms[:, n, :], svec,
                                 start=True, stop=True)
            s12 = single.tile([128, 9], dt.float32)
            nc.scalar.copy(s12, sexp_ps)
            s1b = s12[:, 0:NB]
            s2b = s12[:, NB:NB + NA]

            # ---- fold s into weights: wc = a1*s[j] + a2*s[i] (bf16) ----
            # s1b col l serves chunks m = l (mod 3); s2b col K=5-k serves
            # chunks 3k..3k+2.  ACT muls + one GPSIMD add.
            wc = single.tile([128, NCHUNK, C], dt.bfloat16)
            t1 = single.tile([128, NCHUNK, C], dt.float32)
            t2 = single.tile([128, NCHUNK, C], dt.float32)
            for l in range(NB):
                nc.scalar.mul(t1[:, l::3, :], a1s[:, l::3, :], s1b[:, l:l + 1])
            for k in range(NA):
                ms = slice(3 * k, min(3 * k + 3, NCHUNK))
                nc.scalar.mul(t2[:, ms, :], a2s[:, ms, :],
                              s2b[:, 5 - k:6 - k])
            wcf = wc.rearrange("p a b -> p (a b)")
            t1f = t1.rearrange("p a b -> p (a b)")
            t2f = t2.rearrange("p a b -> p (a b)")
            c1, c2 = 3 * C, 9 * C
            nc.gpsimd.tensor_add(wcf[:, 0:c1], t1f[:, 0:c1], t2f[:, 0:c1])
            nc.gpsimd.tensor_add(wcf[:, c1:c2], t1f[:, c1:c2], t2f[:, c1:c2])
            nc.gpsimd.tensor_add(wcf[:, c2:], t1f[:, c2:], t2f[:, c2:])

            # ---- main sweep: per half, 17 feature TTs feed a column-tiled
            # GEMM (two 512-col subtiles run concurrently per PSUM bank).
            NSH = HALF // NSUB
            for h, hsl in enumerate(hsls):
                banks = [psum.tile([C, NSUB], dt.float32, tag="acc",
                                   name=f"bank{h}_{j}") for j in range(NSH)]
                for m in range(NCHUNK):
                    k, l = divmod(m, 3)
                    f = featp.tile([128, HALF], dt.bfloat16, tag="f")
                    ina = (X2[:, hsl] if k == 0 else
                           bass.AP(tensor=V.tensor,
                                   offset=V.offset + k * P + hsl.start,
                                   ap=[[VP, 128], [1, HALF]]))
                    nc.vector.tensor_mul(
                        f, ina,
                        bass.AP(tensor=V.tensor,
                                offset=V.offset + (NA + l) * P + hsl.start,
                                ap=[[VP, 128], [1, HALF]]))
                    for j in range(NSH):
                        nc.tensor.matmul(banks[j], wc[:, m, :],
                                         f[:, j * NSUB:(j + 1) * NSUB],
                                         start=(m == 0), stop=False)
                # conv_b term: += [B.T; 0].T @ V[b=0 slot] = B @ x (bf16)
                for j in range(NSH):
                    col = h * HALF + j * NSUB
                    nc.tensor.matmul(
                        banks[j], ids,
                        bass.AP(tensor=V.tensor,
                                offset=V.offset + NA * P + col,
                                ap=[[VP, 128], [1, NSUB]]),
                        start=False, stop=True)
                pairs = ([(0, 2), (2, 2)] if h == 0 else
                         [(0, 2), (2, 1), (3, 1)])
                for j0, nj in pairs:
                    col = h * HALF + j0 * NSUB
                    ot = outsp.tile([C, nj * NSUB], dt.float32, tag="o")
                    for jj in range(nj):
                        nc.scalar.copy(ot[:, jj * NSUB:(jj + 1) * NSUB],
                                       banks[j0 + jj])
                    nc.scalar.dma_start(
                        out=out_d.ap()[:, col:col + nj * NSUB], in_=ot)

    nc.compile()
    return nc


def _get_program(niter=None):
    key = ("nc", niter)
    if key not in _CACHE:
        _CACHE[key] = _build_program(niter)
    return _CACHE[key]


def kernel(x, fc1_w, fc2_w, conv_w, conv_b):
    from concourse.bass_utils import run_bass_kernel_spmd

    x = np.asarray(x, np.float32)
    a1, a2, fc1t, fc2t = _host_weights(
        np.asarray(conv_w, np.float32), np.asarray(fc1_w, np.float32),
        np.asarray(fc2_w, np.float32))
    # conv_b contributes sum_i B[c,i]*x_i with B = conv_b.reshape(C, C); the
    # "residual" matmul realizes it with lhsT = [B.T; 0] (identity-init -> +x).
    ident = _host_idb(conv_b)
    nc = _get_program()
    aw = np.ascontiguousarray(np.stack([a1, a2], axis=1))
    in_maps = []
    for b in range(N_CORES):
        xbb2 = x[b].reshape(C, P).astype(ml_dtypes.bfloat16)
        in_maps.append({
            "x": np.ascontiguousarray(np.concatenate([xbb2, xbb2], 0)),
            "aw": aw, "fc1t": fc1t, "fc2t": fc2t, "ident": ident,
            "perm": _host_perm(),
        })
    res = run_bass_kernel_spmd(nc, in_maps, core_ids=list(range(N_CORES)))
    out = np.stack([res.results[b]["out"].reshape(C, H, W)
                    for b in range(N_CORES)], axis=0)
    return out.astype(np.float32)


# revision 34
# speedup vs baseline: 1.2773x; 1.0159x over previous
"""AttentiveManifoldMixer Trainium2 kernel (8-core data parallel over batch).

Math: with W3[c,i,j] = conv_w[c*64+i, j], B = conv_b.reshape(C, C),
  s[b]       = sigmoid(fc2 @ relu(fc1 @ mean_hw(x[b])))
  out[b,c,p] = sum_{i,j} W3[c,i,j] * s[b,j] * x[b,i,p] * x[b,j,p]
               + sum_i B[c,i] * x[b,i,p]

The quadratic form is symmetrized over unordered channel pairs grouped by
cyclic diagonal offset d: chunk m = 3k+l holds lanes q = 64*qhi + qlo with
  i = (qlo + a_k) % 64,   a_k = (64 - 6k) % 64
  j = (qlo + 2l + qhi) % 64
so d = j - i = 6k + 2l + qhi covers 0..33 over 17 chunks (d=32/33 lanes
duplicate at higher mult).  Per-batch weight (W3[c,i,j]s_j + W3[c,j,i]s_i)
/ mult is folded on device.

x arrives host-cast to bf16 and pre-doubled ([x; x], 128 rows); each
column half DMAs it into an SBUF tensor X2 at the head of the SP HWDGE
ring.  Feature operands are 64-partition windows of X2 materialized by
direct SBUF->SBUF partition-window DMAs (two per tile: A_k = (a_k, a_k),
B_l = (2l, 2l+1); A_0 is X2 itself; B_0's low half is an ACT copy of x
that doubles as the SE channel-sum reduction); the copied tiles live in
one packed SBUF tensor V[128, 9, P].  GEMM: 17 bf16 matmuls (K=128, M=64,
N=512) per PSUM bank plus a K=128-padded bf16 conv_b matmul whose rhs is
the B_0 tile (lower rows = x).  The SE sigmoid vector expands to per-lane
columns with 9 tiny on-PE permutation matmuls (no DRAM round-trip); the
weight fold runs on ACT with a 3-way-split final add on GPSIMD so early
chunks' weights unblock the GEMM sooner.  Everything pipelines in two
2048-pixel column halves across the two HWDGE rings.
"""
import sys

sys.path.insert(0, "/opt/trn_rl_repo")

import numpy as np
import ml_dtypes

B, C, H, W = 8, 64, 64, 64
P = H * W                  # 4096 pixels per sample
MID = C // 4
NCHUNK = 17                # feature chunks
NA, NB = 6, 3              # A/B window tiles; chunk m = 3*(m//3) + m%3
NSUB = 512                 # matmul free-dim subtile / psum bank columns
NSPLIT = 2                 # column halves
HALF = P // NSPLIT
NBANK = HALF // (2 * NSUB)  # psum banks per half (2 subtiles per bank)
N_CORES = 8
A_VALS = [64, 58, 52, 46, 40, 34]   # a_k row offset, k=0..5 (64 == 0 mod 64)
NT = NA + NB               # packed window tiles in V

_CACHE = {}


def _lane_maps():
    """Per-lane (i, j, mult): chunk m = 3k+l, lane q = 64*qhi + qlo:
    i = (qlo + a_k) % 64,  j = (qlo + 2l + qhi) % 64."""
    i_idx = np.zeros((NCHUNK, 128), np.int64)
    j_idx = np.zeros((NCHUNK, 128), np.int64)
    for m in range(NCHUNK):
        k, l = divmod(m, 3)
        for q in range(128):
            qhi, qlo = divmod(q, 64)
            i_idx[m, q] = (qlo + A_VALS[k]) % 64
            j_idx[m, q] = (qlo + 2 * l + qhi) % 64
    lo = np.minimum(i_idx, j_idx)
    hi = np.maximum(i_idx, j_idx)
    key = lo * 64 + hi
    _, inv, counts = np.unique(key, return_inverse=True, return_counts=True)
    mult = counts[inv].reshape(key.shape).astype(np.float32)
    return i_idx, j_idx, mult


def _host_weights(conv_w, fc1_w, fc2_w):
    """Pre-gather conv_w into per-lane arrays a1/a2 of shape (128, 17, 64):
    [lane q, chunk m, out-channel c], bf16."""
    w3 = conv_w.reshape(C, C, C)  # [c, i, j]
    i_idx, j_idx, mult = _lane_maps()
    a1 = np.transpose(w3[:, i_idx, j_idx], (2, 1, 0)) / mult.T[:, :, None]
    a2 = np.transpose(w3[:, j_idx, i_idx], (2, 1, 0)) / mult.T[:, :, None]
    diag = (i_idx == j_idx).T  # [q, m]
    a2[diag] = 0.0
    fc1t = (fc1_w.T / float(P)).copy()   # (64, 16): folds the 1/HW of the mean
    fc2t = fc2_w.T.copy()                # (16, 64)
    return (np.ascontiguousarray(a1, ml_dtypes.bfloat16),
            np.ascontiguousarray(a2, ml_dtypes.bfloat16), fc1t, fc2t)


def _host_perm():
    """Permutation lhsTs [64, 9, 128] for the on-PE sigmoid expansion:
    col n<3: s[(qlo + 2n + qhi) % 64]; col 3+K: s[(qlo + 34 + 6K) % 64]."""
    pm = np.zeros((C, 9, 128), np.float32)
    q = np.arange(128)
    qhi, qlo = q // 64, q % 64
    for n in range(9):
        idx = (qlo + 2 * n + qhi) % 64 if n < NB else (qlo + 34 + 6 * (n - NB)) % 64
        pm[idx, n, q] = 1.0
    return pm


def _host_idb(conv_b):
    """conv_b as a K=128-padded bf16 lhsT [B.T; 0] for the residual matmul
    (rhs is the b=0 window tile whose lower 64 rows are x)."""
    bt = np.asarray(conv_b, np.float32).reshape(C, C).T
    idb = np.zeros((128, C), np.float32)
    idb[0:C] = bt
    return np.ascontiguousarray(idb, ml_dtypes.bfloat16)


def _build_program(niter=None):
    """Build the kernel program; with niter, wrap the body in an on-device
    For_i repeat loop (timing variant)."""
    import contextlib

    import concourse.bacc as bacc
    import concourse.bass as bass
    from concourse import mybir
    from concourse.tile import TileContext

    nc = bacc.Bacc("TRN2", target_bir_lowering=False, debug=False)
    dt = mybir.dt

    x_d = nc.dram_tensor("x", [128, P], dt.bfloat16, kind="ExternalInput")
    aw_d = nc.dram_tensor("aw", [128, 2, NCHUNK, C], dt.bfloat16,
                          kind="ExternalInput")
    f1_d = nc.dram_tensor("fc1t", [C, MID], dt.float32, kind="ExternalInput")
    f2_d = nc.dram_tensor("fc2t", [MID, C], dt.float32, kind="ExternalInput")
    id_d = nc.dram_tensor("ident", [128, C], dt.bfloat16, kind="ExternalInput")
    pm_d = nc.dram_tensor("perm", [C, 9, 128], dt.float32, kind="ExternalInput")
    out_d = nc.dram_tensor("out", [C, P], dt.float32, kind="ExternalOutput")

    with TileContext(nc) as tc:
        with tc.tile_pool(name="single", bufs=1) as single, \
             tc.tile_pool(name="dram", bufs=1, space="DRAM") as dpool, \
             tc.tile_pool(name="feat", bufs=16) as featp, \
             tc.tile_pool(name="outs", bufs=6) as outsp, \
             tc.tile_pool(name="psum", bufs=8, space="PSUM") as psum, \
             (tc.For_i(0, niter, 1,
                       hint_engines=(mybir.EngineType.PE,
                                     mybir.EngineType.DVE,
                                     mybir.EngineType.SP,
                                     mybir.EngineType.Pool,
                                     mybir.EngineType.Activation))
              if niter else contextlib.nullcontext()):

            hsls = [slice(i * HALF, (i + 1) * HALF) for i in range(NSPLIT)]
            # Two HWDGE queues: nc.sync (SP) carries the staging writes +
            # window loads + output, nc.scalar (Activation) carries x/weights
            # and the SE-expansion gathers.

            aws = single.tile([128, 2, NCHUNK, C], dt.bfloat16)
            nc.scalar.dma_start(out=aws, in_=aw_d.ap())
            a1s = aws[:, 0]
            a2s = aws[:, 1]
            f1s = single.tile([C, MID], dt.float32)
            nc.scalar.dma_start(out=f1s, in_=f1_d.ap())
            f2s = single.tile([MID, C], dt.float32)
            nc.scalar.dma_start(out=f2s, in_=f2_d.ap())
            ids = single.tile([128, C], dt.bfloat16)
            nc.scalar.dma_start(out=ids, in_=id_d.ap())
            pms = single.tile([C, 9, 128], dt.float32)
            nc.scalar.dma_start(out=pms, in_=pm_d.ap())

            # ---- prestage: load host-doubled [x; x] bf16 (X2), then build
            # the window tiles with direct SBUF->SBUF partition-window DMAs
            # (no DRAM staging).  A_k window (a_k, a_k), a_k in a_low;
            # B_l window (2l, 2l+1); A_0 is X2 itself.
            sums_h = [single.tile([C, 1], dt.float32, name=f"sums{h}")
                      for h in range(NSPLIT)]
            X2 = single.tile([128, P], dt.bfloat16)
            V = single.tile([128, NT, P], dt.bfloat16)
            VP = NT * P  # V per-partition pitch (elements)
            a_low = [0, 58, 52, 46, 40, 34]

            def vdst(qhi, t, hsl):
                return bass.AP(tensor=V.tensor,
                               offset=V.offset + 64 * qhi * VP + t * P
                               + hsl.start,
                               ap=[[VP, 64], [1, HALF]])

            def xwin(row0, hsl):
                return bass.AP(tensor=X2.tensor,
                               offset=X2.offset + row0 * P + hsl.start,
                               ap=[[P, 64], [1, HALF]])

            for h, hsl in enumerate(hsls):
                nc.sync.dma_start(out=X2[:, hsl], in_=x_d.ap()[:, hsl])
                # B0-low (= x) is built by an ACT copy that also emits the
                # SE channel sums; everything else is a window DMA off X2.
                nc.scalar.activation(
                    bass.AP(tensor=V.tensor,
                            offset=V.offset + NA * P + hsl.start,
                            ap=[[VP, C], [1, HALF]]),
                    X2[0:C, hsl],
                    mybir.ActivationFunctionType.Copy,
                    accum_out=sums_h[h])
                nc.sync.dma_start(out=vdst(1, NA, hsl), in_=xwin(1, hsl))
                order = [(NA + l, 2 * l, 2 * l + 1) for l in range(1, NB)] + \
                    [(k, a_low[k], a_low[k]) for k in range(1, NA)]
                for t, wlo, whi in order:
                    nc.sync.dma_start(out=vdst(0, t, hsl),
                                      in_=xwin(wlo, hsl))
                    nc.sync.dma_start(out=vdst(1, t, hsl),
                                      in_=xwin(whi, hsl))

            # ---- SE path: s = sigmoid(fc2t.T @ relu(fc1t.T @ sums)) ----
            ps1 = psum.tile([MID, 1], dt.float32, tag="acc")
            for h in range(NSPLIT):
                nc.tensor.matmul(ps1, f1s, sums_h[h], start=(h == 0),
                                 stop=(h == NSPLIT - 1))
            y1 = single.tile([MID, 1], dt.float32)
            nc.scalar.activation(y1, ps1, mybir.ActivationFunctionType.Relu)
            ps2 = psum.tile([C, 1], dt.float32, tag="acc")
            nc.tensor.matmul(ps2, f2s, y1, start=True, stop=True)
            svec = single.tile([C, 1], dt.float32)
            nc.scalar.activation(svec, ps2, mybir.ActivationFunctionType.Sigmoid)

            # s-expansion on PE: column n of s12 is a permutation-matmul of
            # svec (cols 0..2: s[j] per l; cols 3..8: s[i] per K=5-k).
            sexp_ps = psum.tile([128, 9], dt.float32, tag="acc")
            for n in range(9):
                nc.tensor.matmul(sexp_ps[:, n:n + 1],
                                 pms[:, n, :], svec,
                                 start=True, stop=True)
            s12 = single.tile([128, 9], dt.float32)
            nc.scalar.copy(s12, sexp_ps)
            s1b = s12[:, 0:NB]
            s2b = s12[:, NB:NB + NA]

            # ---- fold s into weights: wc = a1*s[j] + a2*s[i] (bf16) ----
            # s1b col l serves chunks m = l (mod 3); s2b col K=5-k serves
            # chunks 3k..3k+2.  ACT muls + one GPSIMD add.
            wc = single.tile([128, NCHUNK, C], dt.bfloat16)
            t1 = single.tile([128, NCHUNK, C], dt.float32)
            t2 = single.tile([128, NCHUNK, C], dt.float32)
            for l in range(NB):
                nc.scalar.mul(t1[:, l::3, :], a1s[:, l::3, :], s1b[:, l:l + 1])
            for k in range(NA):
                ms = slice(3 * k, min(3 * k + 3, NCHUNK))
                nc.scalar.mul(t2[:, ms, :], a2s[:, ms, :],
                              s2b[:, 5 - k:6 - k])
            wcf = wc.rearrange("p a b -> p (a b)")
            t1f = t1.rearrange("p a b -> p (a b)")
            t2f = t2.rearrange("p a b -> p (a b)")
            c1, c2 = 3 * C, 9 * C
            nc.gpsimd.tensor_add(wcf[:, 0:c1], t1f[:, 0:c1], t2f[:, 0:c1])
            nc.gpsimd.tensor_add(wcf[:, c1:c2], t1f[:, c1:c2], t2f[:, c1:c2])
            nc.gpsimd.tensor_add(wcf[:, c2:], t1f[:, c2:], t2f[:, c2:])

            # ---- main sweep: per half, 17 feature TTs feed a column-tiled
            # GEMM (two 512-col subtiles run concurrently per PSUM bank).
            NSH = HALF // NSUB
            for h, hsl in enumerate(hsls):
                banks = [psum.tile([C, NSUB], dt.float32, tag="acc",
                                   name=f"bank{h}_{j}") for j in range(NSH)]
                for m in range(NCHUNK):
                    k, l = divmod(m, 3)
                    f = featp.tile([128, HALF], dt.bfloat16, tag="f")
                    ina = (X2[:, hsl] if k == 0 else
                           bass.AP(tensor=V.tensor,
                                   offset=V.offset + k * P + hsl.start,
                                   ap=[[VP, 128], [1, HALF]]))
                    nc.vector.tensor_mul(
                        f, ina,
                        bass.AP(tensor=V.tensor,
                                offset=V.offset + (NA + l) * P + hsl.start,
                                ap=[[VP, 128], [1, HALF]]))
                    for j in range(NSH):
                        nc.tensor.matmul(banks[j], wc[:, m, :],
                                         f[:, j * NSUB:(j + 1) * NSUB],
                                         start=(m == 0), stop=False)
                # conv_b term: += [B.T; 0].T @ V[b=0 slot] = B @ x (bf16)
                for j in range(NSH):
                    col = h * HALF + j * NSUB
                    nc.tensor.matmul(
                        banks[j], ids,
                        bass.AP(tensor=V.tensor,
                                offset=V.offset + NA * P + col,
                                ap=[[VP, 128], [1, NSUB]]),
                        start=False, stop=True)
                pairs = ([(0, 2), (2, 2)] if h == 0 else
                         [(0, 2), (2, 1), (3, 1)])
                for j0, nj in pairs:
                    col = h * HALF + j0 * NSUB
                    ot = outsp.tile([C, nj * NSUB], dt.float32, tag="o")
                    for jj in range(nj):
                        # tail banks of the last half evacuate on the DVE,
                        # which is idle after the final feature multiply;
                        # everything else stays on ACT.
                        eng = (nc.vector.tensor_copy
                               if h == 1 and j0 >= 2 else nc.scalar.copy)
                        eng(ot[:, jj * NSUB:(jj + 1) * NSUB],
                            banks[j0 + jj])
                    nc.scalar.dma_start(
                        out=out_d.ap()[:, col:col + nj * NSUB], in_=ot)

    nc.compile()
    return nc


def _get_program(niter=None):
    key = ("nc", niter)
    if key not in _CACHE:
        _CACHE[key] = _build_program(niter)
    return _CACHE[key]


def kernel(x, fc1_w, fc2_w, conv_w, conv_b):
    from concourse.bass_utils import run_bass_kernel_spmd

    x = np.asarray(x, np.float32)
    a1, a2, fc1t, fc2t = _host_weights(
        np.asarray(conv_w, np.float32), np.asarray(fc1_w, np.float32),
        np.asarray(fc2_w, np.float32))
    # conv_b contributes sum_i B[c,i]*x_i with B = conv_b.reshape(C, C); the
    # "residual" matmul realizes it with lhsT = [B.T; 0] (identity-init -> +x).
    ident = _host_idb(conv_b)
    nc = _get_program()
    aw = np.ascontiguousarray(np.stack([a1, a2], axis=1))
    in_maps = []
    for b in range(N_CORES):
        xbb2 = x[b].reshape(C, P).astype(ml_dtypes.bfloat16)
        in_maps.append({
            "x": np.ascontiguousarray(np.concatenate([xbb2, xbb2], 0)),
            "aw": aw, "fc1t": fc1t, "fc2t": fc2t, "ident": ident,
            "perm": _host_perm(),
        })
    res = run_bass_kernel_spmd(nc, in_maps, core_ids=list(range(N_CORES)))
    out = np.stack([res.results[b]["out"].reshape(C, H, W)
                    for b in range(N_CORES)], axis=0)
    return out.astype(np.float32)
